# revision 16
# baseline (speedup 1.0000x reference)
"""Trainium2 Bass kernel for 16-head MHA (B=2, S=2048, D=1024), fp32 I/O.

Sharding: 8 cores = 2 batches x 4 head-groups (4 heads / 256 dims each).
Each core computes q/k/v projections for its head group, attention, and a
partial output projection; the host sums the 4 partial outputs per batch
and adds the output bias.

Device-side layout choices:
  - q,k are produced TRANSPOSED ([dims, seq]) so attention scores come out
    as scores.T ([keys, queries]) with keys on partitions; the AV matmul
    then needs no transpose of the big attention matrix.
  - softmax: no max-subtraction (scores are O(1) by construction; masked
    positions underflow to exp(-1e4)=0). The denominator is obtained for
    free by augmenting V with a ones column; normalization multiplies by
    exp(-ln(denom)) broadcast across partitions.
"""

import os

import numpy as np

NUM_HEADS = 16
D_MODEL = 1024
D_K = 64
B = 2
S = 2048
P = 128
HD = 256  # head-group dims per core (4 heads)
NH = 4  # heads per core
N_CORES = 8
KC8 = D_MODEL // P  # 8 contraction chunks for projections
ST16 = S // P  # 16 seq tiles
MM_DT_NAME = os.environ.get("MHA_MM_DT", "float32r")

_CACHE = {}

# Set by kernel() when PROFILE is truthy: hardware exec time of the slowest
# core in ns, from the NTFF profile.
PROFILE = bool(int(os.environ.get("MHA_PROFILE", "0")))
TRACE_DIR = os.environ.get("MHA_TRACE_DIR", "")
last_exec_time_ns = None
last_results = None


def _build_nc():
    import concourse.mybir as mybir
    from concourse import bacc
    from concourse import tile as tile_mod

    f32 = mybir.dt.float32
    mm_dt = getattr(mybir.dt, MM_DT_NAME)
    # float32r is bit-identical to f32 (PE reads reduced precision), so DRAM
    # params can be declared f32r directly and loaded without a cast. bf16
    # needs a casting DMA (SWDGE / gpsimd path).
    io_dt = mm_dt if MM_DT_NAME == "float32r" else f32
    cast_dma = (MM_DT_NAME != "float32r" and MM_DT_NAME != "float32")
    ldeng = "gpsimd" if cast_dma else "sync"
    # attention operands (q/k for scores, v/attn for AV) run in bf16: the
    # stationaries then qualify for fast-weight-load and halve LDWEIGHTS
    # exposure; projection inputs, z, and the output projection stay mm_dt.
    at_dt = getattr(mybir.dt, os.environ.get("MHA_AT_DT", "bfloat16"))
    AF = mybir.ActivationFunctionType

    def r(ap):
        return ap

    nc = bacc.Bacc()

    xT = nc.declare_dram_parameter("xT", [D_MODEL, S], io_dt, isOutput=False)
    yT = nc.declare_dram_parameter("yT", [D_MODEL, S], io_dt, isOutput=False)
    wqT = nc.declare_dram_parameter("wqT", [D_MODEL, HD], io_dt, isOutput=False)
    wkT = nc.declare_dram_parameter("wkT", [D_MODEL, HD], io_dt, isOutput=False)
    wvT = nc.declare_dram_parameter("wvT", [D_MODEL, HD], io_dt, isOutput=False)
    woT = nc.declare_dram_parameter("woT", [HD, D_MODEL], io_dt, isOutput=False)
    bq = nc.declare_dram_parameter("bq", [P, 2], f32, isOutput=False)
    bk = nc.declare_dram_parameter("bk", [P, 2], f32, isOutput=False)
    bv = nc.declare_dram_parameter("bv", [1, HD], io_dt, isOutput=False)
    maskc = nc.declare_dram_parameter("maskc", [P, ST16], f32, isOutput=False)
    out = nc.declare_dram_parameter("out", [S, D_MODEL], f32, isOutput=True)
    debug = bool(int(os.environ.get("MHA_DEBUG", "0")))
    if debug:
        dbg = {
            "qTo": nc.declare_dram_parameter("qTo", [2 * P, S], f32, isOutput=True),
            "kTo": nc.declare_dram_parameter("kTo", [2 * P, S], f32, isOutput=True),
            "vo": nc.declare_dram_parameter("vo", [ST16 * P, NH * P], f32, isOutput=True),
            "zTo": nc.declare_dram_parameter("zTo", [2 * P, S], f32, isOutput=True),
        }

    with tile_mod.TileContext(nc) as tc:
        with (
            tc.tile_pool(name="const", bufs=1) as cpool,
            tc.tile_pool(name="wpool", bufs=1) as wpool,
            tc.tile_pool(name="qkv", bufs=1) as qkvpool,
        ):
            # ---- persistent tiles ----
            wq_sb = wpool.tile([P, KC8 * HD], mm_dt, tag="wq", name="wq")
            wk_sb = wpool.tile([P, KC8 * HD], mm_dt, tag="wk", name="wk")
            wv_sb = wpool.tile([P, KC8 * HD], mm_dt, tag="wv", name="wv")
            wo_sb = wpool.tile([P, 2 * D_MODEL], mm_dt, tag="wo", name="wo")
            bq_sb = cpool.tile([P, 2], f32, tag="bq", name="bq")
            bk_sb = cpool.tile([P, 2], f32, tag="bk", name="bk")
            bv_sb = cpool.tile([1, HD], mm_dt, tag="bv", name="bv")
            mask_sb = cpool.tile([P, ST16], f32, tag="mask", name="mask")
            ones_sb = cpool.tile([1, P], mm_dt, tag="ones", name="ones")

            qT = [qkvpool.tile([P, S], at_dt, tag=f"qT{m}", name=f"qT{m}") for m in range(2)]
            kT = [qkvpool.tile([P, S], at_dt, tag=f"kT{m}", name=f"kT{m}") for m in range(2)]
            # padded to 128 cols (64 v + 64 ones): the AV stationary is then a
            # full 128-col weight, which enables FWL; the extra psum rows it
            # produces (denominator copies) are never read
            v_aug = [
                qkvpool.tile([P, NH, P], at_dt, tag=f"vaug{st}", name=f"vaug{st}")
                for st in range(ST16)
            ]
            zT = [qkvpool.tile([P, S], mm_dt, tag=f"zT{m}", name=f"zT{m}") for m in range(2)]

            # ---- const / weight loads ----
            # memset lacks an f32r encoding; write the same bits as f32
            def _ms(ap, val):
                if ap.dtype == mybir.dt.float32r:
                    ap = ap.bitcast(f32)
                nc.vector.memset(ap, val)

            _ms(ones_sb[:], 1.0)
            for st in range(ST16):
                _ms(v_aug[st][:], 1.0)
            nc.sync.dma_start(out=bq_sb[:], in_=bq[:])
            nc.sync.dma_start(out=bk_sb[:], in_=bk[:])
            getattr(nc, ldeng).dma_start(out=bv_sb[:], in_=bv[:])
            nc.sync.dma_start(out=mask_sb[:], in_=maskc[:])
            for kc in range(KC8):
                sl = slice(kc * P, (kc + 1) * P)
                csl = slice(kc * HD, (kc + 1) * HD)
                getattr(nc, ldeng).dma_start(out=wq_sb[:, csl], in_=wqT[sl, :])
                getattr(nc, ldeng).dma_start(out=wk_sb[:, csl], in_=wkT[sl, :])
                getattr(nc, ldeng).dma_start(out=wv_sb[:, csl], in_=wvT[sl, :])
            for kc2 in range(2):
                getattr(nc, ldeng).dma_start(
                    out=wo_sb[:, kc2 * D_MODEL : (kc2 + 1) * D_MODEL],
                    in_=woT[kc2 * P : (kc2 + 1) * P, :],
                )

            # ---- phase 1: projections ----
            with (
                nc.named_scope("p1"),
                tc.tile_pool(name="xin", bufs=3) as xin,
                tc.tile_pool(name="ps1", bufs=2, space="PSUM") as ps1,
                tc.tile_pool(name="psv", bufs=4, space="PSUM") as psv,
            ):
                for nh in range(2):
                    nsl = slice(nh * 1024, (nh + 1) * 1024)
                    # q projection (transposed): qT = wq @ x.T
                    pq = [ps1.tile([P, 1024], f32, tag="p1", name="p1") for _ in range(2)]
                    for kc in range(KC8):
                        xc = xin.tile([P, 1024], mm_dt, tag="xc", name="xc")
                        getattr(nc, ldeng).dma_start(
                            out=xc[:], in_=xT[kc * P : (kc + 1) * P, nsl]
                        )
                        for m in range(2):
                            lhs = wq_sb[:, kc * HD + m * P : kc * HD + (m + 1) * P]
                            for n in range(2):
                                nc.tensor.matmul(
                                    pq[m][:, n * 512 : (n + 1) * 512],
                                    r(lhs),
                                    r(xc[:, n * 512 : (n + 1) * 512]),
                                    start=(kc == 0),
                                    stop=(kc == KC8 - 1),
                                )
                    for m in range(2):
                        nc.vector.tensor_scalar_add(
                            qT[m][:, nsl], pq[m][:], bq_sb[:, m : m + 1]
                        )
                    # k (transposed) and v (natural) projections from y
                    pk = [ps1.tile([P, 1024], f32, tag="p1", name="p1") for _ in range(2)]
                    pv = [psv.tile([P, 512], f32, tag="pv", name="pv") for _ in range(4)]
                    for kc in range(KC8):
                        yc = xin.tile([P, 1024], mm_dt, tag="xc", name="xc")
                        getattr(nc, ldeng).dma_start(
                            out=yc[:], in_=yT[kc * P : (kc + 1) * P, nsl]
                        )
                        for m in range(2):
                            lhs = wk_sb[:, kc * HD + m * P : kc * HD + (m + 1) * P]
                            for n in range(2):
                                nc.tensor.matmul(
                                    pk[m][:, n * 512 : (n + 1) * 512],
                                    r(lhs),
                                    r(yc[:, n * 512 : (n + 1) * 512]),
                                    start=(kc == 0),
                                    stop=(kc == KC8 - 1),
                                )
                        for sti in range(8):
                            # two st tiles share one PSUM bank; start=True
                            # clears the WHOLE bank, so only the first tile's
                            # first matmul may carry it (the second tile's
                            # first write lands on cleared has_written bits
                            # and overwrites).
                            nc.tensor.matmul(
                                pv[sti // 2][:, (sti % 2) * 256 : (sti % 2 + 1) * 256],
                                r(yc[:, sti * P : (sti + 1) * P]),
                                r(wv_sb[:, kc * HD : (kc + 1) * HD]),
                                start=(kc == 0 and sti % 2 == 0),
                                stop=False,
                                skip_group_check=True,
                            )
                    for m in range(2):
                        nc.vector.tensor_scalar_add(
                            kT[m][:, nsl], pk[m][:], bk_sb[:, m : m + 1]
                        )
                    for sti in range(8):
                        st = nh * 8 + sti
                        psl = pv[sti // 2][:, (sti % 2) * 256 : (sti % 2 + 1) * 256]
                        # add bias via K=1 matmul (bias varies along free dim)
                        nc.tensor.matmul(
                            psl, ones_sb[:], bv_sb[:], start=False, stop=True
                        )
                        for h in range(NH):
                            nc.vector.tensor_copy(
                                v_aug[st][:, h, 0:D_K],
                                psl[:, h * D_K : (h + 1) * D_K],
                            )

            # ---- phase 2: attention ----
            with (
                nc.named_scope("attn"),
                tc.tile_pool(name="spool", bufs=2, space="PSUM") as spool,
                tc.tile_pool(name="zpool", bufs=2, space="PSUM") as zpool,
                tc.tile_pool(name="apool", bufs=3) as apool,
                tc.tile_pool(name="rows", bufs=2) as rows,
                tc.tile_pool(name="rpool", bufs=2) as rpool,
                tc.tile_pool(name="ztpool", bufs=2) as ztpool,
            ):
                def scores_pair(heads, qh, kc, ps_map):
                    # the two heads' K=64 stationaries sit at partitions 0-63 /
                    # 64-127; explicit tile_position puts them in disjoint PE
                    # row groups so alternating matmuls run concurrently
                    for n in range(2):
                        for h in heads:
                            mb, mo = h // 2, (h % 2) * D_K
                            nc.tensor.matmul(
                                ps_map[h][:, n * 512 : (n + 1) * 512],
                                r(kT[mb][mo : mo + D_K, kc * P : (kc + 1) * P]),
                                r(qT[mb][mo : mo + D_K, qh * 1024 + n * 512 : qh * 1024 + (n + 1) * 512]),
                                start=True,
                                stop=True,
                                tile_position=(mo, 0),
                            )

                # Two heads of a pair are processed interleaved so PE and ACT
                # pipeline; their score matmuls are emitted adjacently and use
                # disjoint PE row groups (K=64 at partitions 0/64), so they
                # run concurrently in the array.
                for qh in range(2):
                    for hp in range(2):
                        heads = (2 * hp, 2 * hp + 1)
                        pz, ps_cur = {}, {}
                        for h in heads:
                            pz[h] = zpool.tile([P, 1024], f32, tag="pz", name="pz")
                        for h in heads:
                            ps_cur[h] = spool.tile([P, 1024], f32, tag="ps", name="ps")
                        scores_pair(heads, qh, 0, ps_cur)
                        for kc in range(ST16):
                            attn = {}
                            for h in heads:
                                attn[h] = apool.tile(
                                    [P, 1024], at_dt, tag="attn", name="attn"
                                )
                                nc.scalar.activation(
                                    attn[h][:],
                                    ps_cur[h][:],
                                    AF.Exp,
                                    bias=mask_sb[:, kc : kc + 1],
                                    scale=0.125,
                                )
                            if kc + 1 < ST16:
                                for h in heads:
                                    ps_cur[h] = spool.tile(
                                        [P, 1024], f32, tag="ps", name="ps"
                                    )
                                scores_pair(heads, qh, kc + 1, ps_cur)
                            for h in heads:
                                for n in range(2):
                                    nc.tensor.matmul(
                                        pz[h][:, n * 512 : (n + 1) * 512],
                                        r(v_aug[kc][:, h, :]),
                                        r(attn[h][:, n * 512 : (n + 1) * 512]),
                                        start=(kc == 0),
                                        stop=(kc == ST16 - 1),
                                    )
                        # normalization: r = exp(-ln(denom)), broadcast, multiply
                        qsl = slice(qh * 1024, (qh + 1) * 1024)
                        for h in heads:
                            mb = h // 2
                            # 1/denominator on DVE (custom op, ~18-bit) keeps
                            # the ACT exclusively on the big exps (one table set)
                            d_row = rows.tile([1, 1024], f32, tag="dr", name="dr")
                            nc.vector.tensor_copy(d_row[:], pz[h][D_K : D_K + 1, :])
                            r_row = rows.tile([1, 1024], f32, tag="rr", name="rr")
                            nc.vector.reciprocal_approx_fast(r_row[:], d_row[:])
                            r_sb = rpool.tile([P, 1024], f32, tag="rb", name="rb")
                            nc.gpsimd.partition_broadcast(
                                r_sb[0:D_K, :], r_row[:], channels=D_K
                            )
                            if h % 2 == 0:
                                nc.vector.tensor_mul(
                                    zT[mb][0:D_K, qsl], pz[h][0:D_K, :], r_sb[0:D_K, :]
                                )
                            else:
                                zt = ztpool.tile(
                                    [D_K, 1024], mm_dt, tag="zt", name="zt"
                                )
                                nc.vector.tensor_mul(
                                    zt[:], pz[h][0:D_K, :], r_sb[0:D_K, :]
                                )
                                nc.sync.dma_start(out=zT[mb][D_K:P, qsl], in_=zt[:])

            # ---- phase 3: output projection (partial; host sums groups) ----
            with (
                nc.named_scope("oproj"),
                tc.tile_pool(name="opool", bufs=4, space="PSUM") as opool,
                tc.tile_pool(name="obuf", bufs=3) as obuf,
            ):
                for st in range(ST16):
                    osb = obuf.tile([P, D_MODEL], f32, tag="osb", name="osb")
                    for n in range(2):
                        po = opool.tile([P, 512], f32, tag="po", name="po")
                        for kc2 in range(2):
                            nc.tensor.matmul(
                                po[:],
                                r(zT[kc2][:, st * P : (st + 1) * P]),
                                r(wo_sb[:, kc2 * D_MODEL + n * 512 : kc2 * D_MODEL + (n + 1) * 512]),
                                start=(kc2 == 0),
                                stop=(kc2 == 1),
                            )
                        nc.vector.tensor_copy(osb[:, n * 512 : (n + 1) * 512], po[:])
                    nc.sync.dma_start(out=out[st * P : (st + 1) * P, :], in_=osb[:])
            if debug:
                for m in range(2):
                    nc.gpsimd.dma_start(out=dbg["qTo"][m * P : (m + 1) * P, :], in_=qT[m][:])
                    nc.gpsimd.dma_start(out=dbg["kTo"][m * P : (m + 1) * P, :], in_=kT[m][:])
                    nc.gpsimd.dma_start(out=dbg["zTo"][m * P : (m + 1) * P, :], in_=zT[m][:])
                for st in range(ST16):
                    nc.gpsimd.dma_start(
                        out=dbg["vo"][st * P : (st + 1) * P, :],
                        in_=v_aug[st][:].rearrange("p a b -> p (a b)"),
                    )

    nc.compile()
    return nc


def _get_nc():
    if "nc" not in _CACHE:
        _CACHE["nc"] = _build_nc()
    return _CACHE["nc"]


def _ensure_ntff_hook():
    """Provide antenv.axon_hooks (absent in this container) so that
    run_bass_kernel_spmd(trace=True) can NTFF-profile via ctypes."""
    import sys
    import types

    try:
        from antenv.axon_hooks import get_axon_ntff_profile_hook  # noqa: F401

        return
    except ImportError:
        pass
    try:
        from trn_agent_boot.trn_boot import _ntff_profile_via_ctypes

        hook = _ntff_profile_via_ctypes("/opt/axon/libaxon_pjrt.so")
    except Exception:
        hook = None
    mod = types.ModuleType("antenv.axon_hooks")
    state = {"hook": hook}
    mod.get_axon_ntff_profile_hook = lambda: state["hook"]
    mod.set_axon_ntff_profile_hook = lambda h: state.__setitem__("hook", h)
    sys.modules["antenv.axon_hooks"] = mod
    import antenv

    antenv.axon_hooks = mod


def kernel(x, y, mask, wq, bq, wk, bk, wv, bv, wo, bo):
    global last_exec_time_ns, last_results
    from concourse.bass_utils import run_bass_kernel_spmd

    if PROFILE:
        _ensure_ntff_hook()

    x = np.asarray(x, dtype=np.float32)
    y = np.asarray(y, dtype=np.float32)
    mask = np.asarray(mask, dtype=np.float32)
    wq, bq = np.asarray(wq, np.float32), np.asarray(bq, np.float32)
    wk, bk = np.asarray(wk, np.float32), np.asarray(bk, np.float32)
    wv, bv = np.asarray(wv, np.float32), np.asarray(bv, np.float32)
    wo, bo = np.asarray(wo, np.float32), np.asarray(bo, np.float32)

    nc = _get_nc()

    xTs = [np.ascontiguousarray(x[b].T) for b in range(B)]
    yTs = [np.ascontiguousarray(y[b].T) for b in range(B)]
    maskcs = [
        np.ascontiguousarray(((1.0 - mask[b]) * -10000.0).reshape(ST16, P).T)
        for b in range(B)
    ]
    in_maps = []
    for c in range(N_CORES):
        b, g = c // 4, c % 4
        sl = slice(g * HD, (g + 1) * HD)
        in_maps.append(
            {
                "xT": xTs[b],
                "yT": yTs[b],
                "wqT": np.ascontiguousarray(wq[sl, :].T),
                "wkT": np.ascontiguousarray(wk[sl, :].T),
                "wvT": np.ascontiguousarray(wv[sl, :].T),
                "woT": np.ascontiguousarray(wo[:, sl].T),
                "bq": np.ascontiguousarray(bq[sl].reshape(2, P).T),
                "bk": np.ascontiguousarray(bk[sl].reshape(2, P).T),
                "bv": np.ascontiguousarray(bv[sl].reshape(1, HD)),
                "maskc": maskcs[b],
            }
        )

    kwargs = {}
    if PROFILE:
        kwargs["trace"] = True
        if TRACE_DIR:
            os.makedirs(TRACE_DIR, exist_ok=True)
            kwargs["tmpdir"] = TRACE_DIR
    res = run_bass_kernel_spmd(nc, in_maps, list(range(N_CORES)), **kwargs)
    last_results = res
    last_exec_time_ns = res.exec_time_ns

    out = np.empty((B, S, D_MODEL), np.float32)
    for b in range(B):
        acc = res.results[b * 4]["out"].astype(np.float32)
        for g in range(1, 4):
            acc = acc + res.results[b * 4 + g]["out"]
        out[b] = acc + bo[None, :]
    return out


# revision 19
# speedup vs baseline: 1.3049x; 1.3049x over previous
"""Trainium2 Bass kernel for 16-head MHA (B=2, S=2048, D=1024), fp32 I/O.

Sharding: 8 cores = 2 batches x 4 head-groups (4 heads / 256 dims each).
Each core computes q/k/v projections for its head group, attention, and a
partial output projection; the host sums the 4 partial outputs per batch
and adds the output bias.

Device-side layout choices:
  - q,k are produced TRANSPOSED ([dims, seq]) so attention scores come out
    as scores.T ([keys, queries]) with keys on partitions; the AV matmul
    then needs no transpose of the big attention matrix.
  - softmax: no max-subtraction (scores are O(1) by construction; masked
    positions underflow to exp(-1e4)=0). The denominator is obtained for
    free by augmenting V with a ones column; normalization multiplies by
    exp(-ln(denom)) broadcast across partitions.
"""

import os

import numpy as np

NUM_HEADS = 16
D_MODEL = 1024
D_K = 64
B = 2
S = 2048
P = 128
HD = 256  # head-group dims per core (4 heads)
NH = 4  # heads per core
N_CORES = 8
KC8 = D_MODEL // P  # 8 contraction chunks for projections
ST16 = S // P  # 16 seq tiles
MM_DT_NAME = os.environ.get("MHA_MM_DT", "float32r")

_CACHE = {}

# Set by kernel() when PROFILE is truthy: hardware exec time of the slowest
# core in ns, from the NTFF profile.
PROFILE = bool(int(os.environ.get("MHA_PROFILE", "0")))
TRACE_DIR = os.environ.get("MHA_TRACE_DIR", "")
last_exec_time_ns = None
last_results = None


def _build_nc():
    import concourse.mybir as mybir
    from concourse import bacc
    from concourse import tile as tile_mod

    f32 = mybir.dt.float32
    mm_dt = getattr(mybir.dt, MM_DT_NAME)
    # float32r is bit-identical to f32 (PE reads reduced precision), so DRAM
    # params can be declared f32r directly and loaded without a cast. bf16
    # needs a casting DMA (SWDGE / gpsimd path).
    io_dt = mm_dt if MM_DT_NAME == "float32r" else f32
    cast_dma = (MM_DT_NAME != "float32r" and MM_DT_NAME != "float32")
    ldeng = "gpsimd" if cast_dma else "sync"
    # attention operands (q/k for scores, v/attn for AV) run in bf16: the
    # stationaries then qualify for fast-weight-load and halve LDWEIGHTS
    # exposure; projection inputs, z, and the output projection stay mm_dt.
    at_dt = getattr(mybir.dt, os.environ.get("MHA_AT_DT", "bfloat16"))
    AF = mybir.ActivationFunctionType

    def r(ap):
        return ap

    nc = bacc.Bacc()

    xT = nc.declare_dram_parameter("xT", [D_MODEL, S], io_dt, isOutput=False)
    yT = nc.declare_dram_parameter("yT", [D_MODEL, S], io_dt, isOutput=False)
    wqT = nc.declare_dram_parameter("wqT", [D_MODEL, HD], io_dt, isOutput=False)
    wkT = nc.declare_dram_parameter("wkT", [D_MODEL, HD], io_dt, isOutput=False)
    wvT = nc.declare_dram_parameter("wvT", [D_MODEL, HD], io_dt, isOutput=False)
    woT = nc.declare_dram_parameter("woT", [HD, D_MODEL], io_dt, isOutput=False)
    bq = nc.declare_dram_parameter("bq", [P, 2], f32, isOutput=False)
    bk = nc.declare_dram_parameter("bk", [P, 2], f32, isOutput=False)
    bv = nc.declare_dram_parameter("bv", [1, HD], io_dt, isOutput=False)
    maskc = nc.declare_dram_parameter("maskc", [P, ST16], f32, isOutput=False)
    out = nc.declare_dram_parameter("out", [S, D_MODEL], f32, isOutput=True)
    debug = bool(int(os.environ.get("MHA_DEBUG", "0")))
    if debug:
        dbg = {
            "qTo": nc.declare_dram_parameter("qTo", [2 * P, S], f32, isOutput=True),
            "kTo": nc.declare_dram_parameter("kTo", [2 * P, S], f32, isOutput=True),
            "vo": nc.declare_dram_parameter("vo", [ST16 * P, NH * P], f32, isOutput=True),
            "zTo": nc.declare_dram_parameter("zTo", [2 * P, S], f32, isOutput=True),
        }

    with tile_mod.TileContext(nc) as tc:
        with (
            tc.tile_pool(name="const", bufs=1) as cpool,
            tc.tile_pool(name="wpool", bufs=1) as wpool,
            tc.tile_pool(name="qkv", bufs=1) as qkvpool,
        ):
            # ---- persistent tiles ----
            wq_sb = wpool.tile([P, KC8 * HD], mm_dt, tag="wq", name="wq")
            wk_sb = wpool.tile([P, KC8 * HD], mm_dt, tag="wk", name="wk")
            wv_sb = wpool.tile([P, KC8 * HD], mm_dt, tag="wv", name="wv")
            wo_sb = wpool.tile([P, 2 * D_MODEL], mm_dt, tag="wo", name="wo")
            bq_sb = cpool.tile([P, 2], f32, tag="bq", name="bq")
            bk_sb = cpool.tile([P, 2], f32, tag="bk", name="bk")
            bv_sb = cpool.tile([1, HD], mm_dt, tag="bv", name="bv")
            mask_sb = cpool.tile([P, ST16], f32, tag="mask", name="mask")
            ones_sb = cpool.tile([1, P], mm_dt, tag="ones", name="ones")

            qT = [qkvpool.tile([P, S], at_dt, tag=f"qT{m}", name=f"qT{m}") for m in range(2)]
            kT = [qkvpool.tile([P, S], at_dt, tag=f"kT{m}", name=f"kT{m}") for m in range(2)]
            # padded to 128 cols (64 v + 64 ones): the AV stationary is then a
            # full 128-col weight, which enables FWL; the extra psum rows it
            # produces (denominator copies) are never read
            v_aug = [
                qkvpool.tile([P, NH, P], at_dt, tag=f"vaug{st}", name=f"vaug{st}")
                for st in range(ST16)
            ]
            zT = [qkvpool.tile([P, S], mm_dt, tag=f"zT{m}", name=f"zT{m}") for m in range(2)]

            # ---- const / weight loads ----
            # memset lacks an f32r encoding; write the same bits as f32
            def _ms(ap, val):
                if ap.dtype == mybir.dt.float32r:
                    ap = ap.bitcast(f32)
                nc.vector.memset(ap, val)

            _ms(ones_sb[:], 1.0)
            for st in range(ST16):
                _ms(v_aug[st][:], 1.0)
            nc.sync.dma_start(out=bq_sb[:], in_=bq[:])
            nc.sync.dma_start(out=bk_sb[:], in_=bk[:])
            getattr(nc, ldeng).dma_start(out=bv_sb[:], in_=bv[:])
            nc.sync.dma_start(out=mask_sb[:], in_=maskc[:])
            for kc in range(KC8):
                sl = slice(kc * P, (kc + 1) * P)
                csl = slice(kc * HD, (kc + 1) * HD)
                getattr(nc, ldeng).dma_start(out=wq_sb[:, csl], in_=wqT[sl, :])
                getattr(nc, ldeng).dma_start(out=wk_sb[:, csl], in_=wkT[sl, :])
                getattr(nc, ldeng).dma_start(out=wv_sb[:, csl], in_=wvT[sl, :])
            for kc2 in range(2):
                getattr(nc, ldeng).dma_start(
                    out=wo_sb[:, kc2 * D_MODEL : (kc2 + 1) * D_MODEL],
                    in_=woT[kc2 * P : (kc2 + 1) * P, :],
                )

            # ---- phase 1: projections ----
            with (
                nc.named_scope("p1"),
                tc.tile_pool(name="xin", bufs=6) as xin,
                tc.tile_pool(name="ps1", bufs=2, space="PSUM") as ps1,
                tc.tile_pool(name="psv", bufs=4, space="PSUM") as psv,
            ):
                for nh in range(2):
                    nsl = slice(nh * 1024, (nh + 1) * 1024)
                    # q projection (transposed): qT = wq @ x.T
                    pq = [ps1.tile([P, 1024], f32, tag="p1", name="p1") for _ in range(2)]
                    for kc in range(KC8):
                        xc = xin.tile([P, 1024], mm_dt, tag="xc", name="xc")
                        getattr(nc, ldeng).dma_start(
                            out=xc[:], in_=xT[kc * P : (kc + 1) * P, nsl]
                        )
                        for m in range(2):
                            lhs = wq_sb[:, kc * HD + m * P : kc * HD + (m + 1) * P]
                            for n in range(2):
                                nc.tensor.matmul(
                                    pq[m][:, n * 512 : (n + 1) * 512],
                                    r(lhs),
                                    r(xc[:, n * 512 : (n + 1) * 512]),
                                    start=(kc == 0),
                                    stop=(kc == KC8 - 1),
                                )
                    for m in range(2):
                        nc.vector.tensor_scalar_add(
                            qT[m][:, nsl], pq[m][:], bq_sb[:, m : m + 1]
                        )
                    # k (transposed) and v (natural) projections from y
                    pk = [ps1.tile([P, 1024], f32, tag="p1", name="p1") for _ in range(2)]
                    pv = [psv.tile([P, 512], f32, tag="pv", name="pv") for _ in range(4)]
                    for kc in range(KC8):
                        yc = xin.tile([P, 1024], mm_dt, tag="xc", name="xc")
                        getattr(nc, ldeng).dma_start(
                            out=yc[:], in_=yT[kc * P : (kc + 1) * P, nsl]
                        )
                        for m in range(2):
                            lhs = wk_sb[:, kc * HD + m * P : kc * HD + (m + 1) * P]
                            for n in range(2):
                                nc.tensor.matmul(
                                    pk[m][:, n * 512 : (n + 1) * 512],
                                    r(lhs),
                                    r(yc[:, n * 512 : (n + 1) * 512]),
                                    start=(kc == 0),
                                    stop=(kc == KC8 - 1),
                                )
                        for sti in range(8):
                            # two st tiles share one PSUM bank; start=True
                            # clears the WHOLE bank, so only the first tile's
                            # first matmul may carry it (the second tile's
                            # first write lands on cleared has_written bits
                            # and overwrites).
                            nc.tensor.matmul(
                                pv[sti // 2][:, (sti % 2) * 256 : (sti % 2 + 1) * 256],
                                r(yc[:, sti * P : (sti + 1) * P]),
                                r(wv_sb[:, kc * HD : (kc + 1) * HD]),
                                start=(kc == 0 and sti % 2 == 0),
                                stop=False,
                                skip_group_check=True,
                            )
                    for m in range(2):
                        nc.vector.tensor_scalar_add(
                            kT[m][:, nsl], pk[m][:], bk_sb[:, m : m + 1]
                        )
                    for sti in range(8):
                        st = nh * 8 + sti
                        psl = pv[sti // 2][:, (sti % 2) * 256 : (sti % 2 + 1) * 256]
                        # add bias via K=1 matmul (bias varies along free dim)
                        nc.tensor.matmul(
                            psl, ones_sb[:], bv_sb[:], start=False, stop=True
                        )
                        for h in range(NH):
                            nc.vector.tensor_copy(
                                v_aug[st][:, h, 0:D_K],
                                psl[:, h * D_K : (h + 1) * D_K],
                            )

            # ---- phase 2: attention ----
            with (
                nc.named_scope("attn"),
                tc.tile_pool(name="spool", bufs=2, space="PSUM") as spool,
                tc.tile_pool(name="zpool", bufs=2, space="PSUM") as zpool,
                tc.tile_pool(name="apool", bufs=3) as apool,
                tc.tile_pool(name="rows", bufs=2) as rows,
                tc.tile_pool(name="rpool", bufs=2) as rpool,
                tc.tile_pool(name="ztpool", bufs=2) as ztpool,
            ):
                def scores_pair(heads, qh, kc, ps_map):
                    # the two heads' K=64 stationaries sit at partitions 0-63 /
                    # 64-127; explicit tile_position puts them in disjoint PE
                    # row groups so alternating matmuls run concurrently
                    for n in range(2):
                        for h in heads:
                            mb, mo = h // 2, (h % 2) * D_K
                            nc.tensor.matmul(
                                ps_map[h][:, n * 512 : (n + 1) * 512],
                                r(kT[mb][mo : mo + D_K, kc * P : (kc + 1) * P]),
                                r(qT[mb][mo : mo + D_K, qh * 1024 + n * 512 : qh * 1024 + (n + 1) * 512]),
                                start=True,
                                stop=True,
                                tile_position=(mo, 0),
                            )

                # Two heads of a pair are processed interleaved so PE and ACT
                # pipeline; their score matmuls are emitted adjacently and use
                # disjoint PE row groups (K=64 at partitions 0/64), so they
                # run concurrently in the array.
                for qh in range(2):
                    for hp in range(2):
                        heads = (2 * hp, 2 * hp + 1)
                        pz, ps_cur = {}, {}
                        for h in heads:
                            pz[h] = zpool.tile([P, 1024], f32, tag="pz", name="pz")
                        for h in heads:
                            ps_cur[h] = spool.tile([P, 1024], f32, tag="ps", name="ps")
                        scores_pair(heads, qh, 0, ps_cur)

                        def av(h, kc, at):
                            for n in range(2):
                                nc.tensor.matmul(
                                    pz[h][:, n * 512 : (n + 1) * 512],
                                    r(v_aug[kc][:, h, :]),
                                    r(at[:, n * 512 : (n + 1) * 512]),
                                    start=(kc == 0),
                                    stop=(kc == ST16 - 1),
                                )

                        # Round structure: exps first, then the PREVIOUS
                        # round's AVs (they hide under the exps on PE), then
                        # the interleaved scores quad for the next round.
                        prev_attn = None
                        for kc in range(ST16):
                            attn = {}
                            for h in heads:
                                attn[h] = apool.tile(
                                    [P, 1024], at_dt, tag="attn", name="attn"
                                )
                                nc.scalar.activation(
                                    attn[h][:],
                                    ps_cur[h][:],
                                    AF.Exp,
                                    bias=mask_sb[:, kc : kc + 1],
                                    scale=0.125,
                                )
                            if prev_attn is not None:
                                for h in heads:
                                    av(h, kc - 1, prev_attn[h])
                            if kc + 1 < ST16:
                                for h in heads:
                                    ps_cur[h] = spool.tile(
                                        [P, 1024], f32, tag="ps", name="ps"
                                    )
                                scores_pair(heads, qh, kc + 1, ps_cur)
                            prev_attn = attn
                        for h in heads:
                            av(h, ST16 - 1, prev_attn[h])
                        # normalization: z/denominator. First evacuate psum to
                        # SBUF with one copy (frees the z banks fast), then
                        # reciprocal + broadcast + multiply off the hot path.
                        qsl = slice(qh * 1024, (qh + 1) * 1024)
                        for h in heads:
                            mb = h // 2
                            z_sb = ztpool.tile(
                                [D_K + 1, 1024], f32, tag="zsb", name="zsb"
                            )
                            nc.vector.tensor_copy(z_sb[:], pz[h][0 : D_K + 1, :])
                            # the custom DVE op mis-reads inputs at partition
                            # offset 64; re-base the denominator row to
                            # partition 0 first
                            d_row = rows.tile([1, 1024], f32, tag="dr", name="dr")
                            nc.vector.tensor_copy(d_row[:], z_sb[D_K : D_K + 1, :])
                            r_row = rows.tile([1, 1024], f32, tag="rr", name="rr")
                            nc.vector.reciprocal_approx_fast(r_row[:], d_row[:])
                            r_sb = rpool.tile([P, 1024], f32, tag="rb", name="rb")
                            nc.gpsimd.partition_broadcast(
                                r_sb[0:D_K, :], r_row[:], channels=D_K
                            )
                            if h % 2 == 0:
                                nc.vector.tensor_mul(
                                    zT[mb][0:D_K, qsl], z_sb[0:D_K, :], r_sb[0:D_K, :]
                                )
                            else:
                                zt = ztpool.tile(
                                    [D_K, 1024], mm_dt, tag="zt", name="zt"
                                )
                                nc.vector.tensor_mul(
                                    zt[:], z_sb[0:D_K, :], r_sb[0:D_K, :]
                                )
                                nc.sync.dma_start(out=zT[mb][D_K:P, qsl], in_=zt[:])

            # ---- phase 3: output projection (partial; host sums groups) ----
            with (
                nc.named_scope("oproj"),
                tc.tile_pool(name="opool", bufs=4, space="PSUM") as opool,
                tc.tile_pool(name="obuf", bufs=3) as obuf,
            ):
                for st in range(ST16):
                    osb = obuf.tile([P, D_MODEL], f32, tag="osb", name="osb")
                    for n in range(2):
                        po = opool.tile([P, 512], f32, tag="po", name="po")
                        for kc2 in range(2):
                            nc.tensor.matmul(
                                po[:],
                                r(zT[kc2][:, st * P : (st + 1) * P]),
                                r(wo_sb[:, kc2 * D_MODEL + n * 512 : kc2 * D_MODEL + (n + 1) * 512]),
                                start=(kc2 == 0),
                                stop=(kc2 == 1),
                            )
                        nc.vector.tensor_copy(osb[:, n * 512 : (n + 1) * 512], po[:])
                    nc.sync.dma_start(out=out[st * P : (st + 1) * P, :], in_=osb[:])
            if debug:
                for m in range(2):
                    nc.gpsimd.dma_start(out=dbg["qTo"][m * P : (m + 1) * P, :], in_=qT[m][:])
                    nc.gpsimd.dma_start(out=dbg["kTo"][m * P : (m + 1) * P, :], in_=kT[m][:])
                    nc.gpsimd.dma_start(out=dbg["zTo"][m * P : (m + 1) * P, :], in_=zT[m][:])
                for st in range(ST16):
                    nc.gpsimd.dma_start(
                        out=dbg["vo"][st * P : (st + 1) * P, :],
                        in_=v_aug[st][:].rearrange("p a b -> p (a b)"),
                    )

    nc.compile()
    return nc


def _get_nc():
    if "nc" not in _CACHE:
        _CACHE["nc"] = _build_nc()
    return _CACHE["nc"]


def _ensure_ntff_hook():
    """Provide antenv.axon_hooks (absent in this container) so that
    run_bass_kernel_spmd(trace=True) can NTFF-profile via ctypes."""
    import sys
    import types

    try:
        from antenv.axon_hooks import get_axon_ntff_profile_hook  # noqa: F401

        return
    except ImportError:
        pass
    try:
        from trn_agent_boot.trn_boot import _ntff_profile_via_ctypes

        hook = _ntff_profile_via_ctypes("/opt/axon/libaxon_pjrt.so")
    except Exception:
        hook = None
    mod = types.ModuleType("antenv.axon_hooks")
    state = {"hook": hook}
    mod.get_axon_ntff_profile_hook = lambda: state["hook"]
    mod.set_axon_ntff_profile_hook = lambda h: state.__setitem__("hook", h)
    sys.modules["antenv.axon_hooks"] = mod
    import antenv

    antenv.axon_hooks = mod


def kernel(x, y, mask, wq, bq, wk, bk, wv, bv, wo, bo):
    global last_exec_time_ns, last_results
    from concourse.bass_utils import run_bass_kernel_spmd

    if PROFILE:
        _ensure_ntff_hook()

    x = np.asarray(x, dtype=np.float32)
    y = np.asarray(y, dtype=np.float32)
    mask = np.asarray(mask, dtype=np.float32)
    wq, bq = np.asarray(wq, np.float32), np.asarray(bq, np.float32)
    wk, bk = np.asarray(wk, np.float32), np.asarray(bk, np.float32)
    wv, bv = np.asarray(wv, np.float32), np.asarray(bv, np.float32)
    wo, bo = np.asarray(wo, np.float32), np.asarray(bo, np.float32)

    nc = _get_nc()

    xTs = [np.ascontiguousarray(x[b].T) for b in range(B)]
    yTs = [np.ascontiguousarray(y[b].T) for b in range(B)]
    maskcs = [
        np.ascontiguousarray(((1.0 - mask[b]) * -10000.0).reshape(ST16, P).T)
        for b in range(B)
    ]
    in_maps = []
    for c in range(N_CORES):
        b, g = c // 4, c % 4
        sl = slice(g * HD, (g + 1) * HD)
        in_maps.append(
            {
                "xT": xTs[b],
                "yT": yTs[b],
                "wqT": np.ascontiguousarray(wq[sl, :].T),
                "wkT": np.ascontiguousarray(wk[sl, :].T),
                "wvT": np.ascontiguousarray(wv[sl, :].T),
                "woT": np.ascontiguousarray(wo[:, sl].T),
                "bq": np.ascontiguousarray(bq[sl].reshape(2, P).T),
                "bk": np.ascontiguousarray(bk[sl].reshape(2, P).T),
                "bv": np.ascontiguousarray(bv[sl].reshape(1, HD)),
                "maskc": maskcs[b],
            }
        )

    kwargs = {}
    if PROFILE:
        kwargs["trace"] = True
        if TRACE_DIR:
            os.makedirs(TRACE_DIR, exist_ok=True)
            kwargs["tmpdir"] = TRACE_DIR
    res = run_bass_kernel_spmd(nc, in_maps, list(range(N_CORES)), **kwargs)
    last_results = res
    last_exec_time_ns = res.exec_time_ns

    out = np.empty((B, S, D_MODEL), np.float32)
    for b in range(B):
        acc = res.results[b * 4]["out"].astype(np.float32)
        for g in range(1, 4):
            acc = acc + res.results[b * 4 + g]["out"]
        out[b] = acc + bo[None, :]
    return out


# revision 20
# speedup vs baseline: 1.3146x; 1.0074x over previous
"""Trainium2 Bass kernel for 16-head MHA (B=2, S=2048, D=1024), fp32 I/O.

Sharding: 8 cores = 2 batches x 4 head-groups (4 heads / 256 dims each).
Each core computes q/k/v projections for its head group, attention, and a
partial output projection; the host sums the 4 partial outputs per batch
and adds the output bias.

Device-side layout choices:
  - q,k are produced TRANSPOSED ([dims, seq]) so attention scores come out
    as scores.T ([keys, queries]) with keys on partitions; the AV matmul
    then needs no transpose of the big attention matrix.
  - softmax: no max-subtraction (scores are O(1) by construction; masked
    positions underflow to exp(-1e4)=0). The denominator is obtained for
    free by augmenting V with a ones column; normalization multiplies by
    exp(-ln(denom)) broadcast across partitions.
"""

import os

import numpy as np

NUM_HEADS = 16
D_MODEL = 1024
D_K = 64
B = 2
S = 2048
P = 128
HD = 256  # head-group dims per core (4 heads)
NH = 4  # heads per core
N_CORES = 8
KC8 = D_MODEL // P  # 8 contraction chunks for projections
ST16 = S // P  # 16 seq tiles
MM_DT_NAME = os.environ.get("MHA_MM_DT", "float32r")

_CACHE = {}

# Set by kernel() when PROFILE is truthy: hardware exec time of the slowest
# core in ns, from the NTFF profile.
PROFILE = bool(int(os.environ.get("MHA_PROFILE", "0")))
TRACE_DIR = os.environ.get("MHA_TRACE_DIR", "")
last_exec_time_ns = None
last_results = None


def _build_nc():
    import concourse.mybir as mybir
    from concourse import bacc
    from concourse import tile as tile_mod

    f32 = mybir.dt.float32
    mm_dt = getattr(mybir.dt, MM_DT_NAME)
    # float32r is bit-identical to f32 (PE reads reduced precision), so DRAM
    # params can be declared f32r directly and loaded without a cast. bf16
    # needs a casting DMA (SWDGE / gpsimd path).
    io_dt = mm_dt if MM_DT_NAME == "float32r" else f32
    cast_dma = (MM_DT_NAME != "float32r" and MM_DT_NAME != "float32")
    ldeng = "gpsimd" if cast_dma else "sync"
    # attention operands (q/k for scores, v/attn for AV) run in bf16: the
    # stationaries then qualify for fast-weight-load and halve LDWEIGHTS
    # exposure; projection inputs, z, and the output projection stay mm_dt.
    at_dt = getattr(mybir.dt, os.environ.get("MHA_AT_DT", "bfloat16"))
    AF = mybir.ActivationFunctionType

    def r(ap):
        return ap

    nc = bacc.Bacc()

    xT = nc.declare_dram_parameter("xT", [D_MODEL, S], io_dt, isOutput=False)
    yT = nc.declare_dram_parameter("yT", [D_MODEL, S], io_dt, isOutput=False)
    wqT = nc.declare_dram_parameter("wqT", [D_MODEL, HD], io_dt, isOutput=False)
    wkT = nc.declare_dram_parameter("wkT", [D_MODEL, HD], io_dt, isOutput=False)
    wvT = nc.declare_dram_parameter("wvT", [D_MODEL, HD], io_dt, isOutput=False)
    woT = nc.declare_dram_parameter("woT", [HD, D_MODEL], io_dt, isOutput=False)
    bq = nc.declare_dram_parameter("bq", [P, 2], f32, isOutput=False)
    bk = nc.declare_dram_parameter("bk", [P, 2], f32, isOutput=False)
    bv = nc.declare_dram_parameter("bv", [1, HD], io_dt, isOutput=False)
    maskc = nc.declare_dram_parameter("maskc", [P, ST16], f32, isOutput=False)
    out = nc.declare_dram_parameter("out", [S, D_MODEL], f32, isOutput=True)
    debug = bool(int(os.environ.get("MHA_DEBUG", "0")))
    if debug:
        dbg = {
            "qTo": nc.declare_dram_parameter("qTo", [2 * P, S], f32, isOutput=True),
            "kTo": nc.declare_dram_parameter("kTo", [2 * P, S], f32, isOutput=True),
            "vo": nc.declare_dram_parameter("vo", [ST16 * P, NH * P], f32, isOutput=True),
            "zTo": nc.declare_dram_parameter("zTo", [2 * P, S], f32, isOutput=True),
        }

    with tile_mod.TileContext(nc) as tc:
        with (
            tc.tile_pool(name="const", bufs=1) as cpool,
            tc.tile_pool(name="wpool", bufs=1) as wpool,
            tc.tile_pool(name="qkv", bufs=1) as qkvpool,
        ):
            # ---- persistent tiles ----
            wq_sb = wpool.tile([P, KC8 * HD], mm_dt, tag="wq", name="wq")
            wk_sb = wpool.tile([P, KC8 * HD], mm_dt, tag="wk", name="wk")
            wv_sb = wpool.tile([P, KC8 * HD], mm_dt, tag="wv", name="wv")
            wo_sb = wpool.tile([P, 2 * D_MODEL], mm_dt, tag="wo", name="wo")
            bq_sb = cpool.tile([P, 2], f32, tag="bq", name="bq")
            bk_sb = cpool.tile([P, 2], f32, tag="bk", name="bk")
            bv_sb = cpool.tile([1, HD], mm_dt, tag="bv", name="bv")
            mask_sb = cpool.tile([P, ST16], f32, tag="mask", name="mask")
            ones_sb = cpool.tile([1, P], mm_dt, tag="ones", name="ones")

            qT = [qkvpool.tile([P, S], at_dt, tag=f"qT{m}", name=f"qT{m}") for m in range(2)]
            kT = [qkvpool.tile([P, S], at_dt, tag=f"kT{m}", name=f"kT{m}") for m in range(2)]
            # padded to 128 cols (64 v + 64 ones): the AV stationary is then a
            # full 128-col weight, which enables FWL; the extra psum rows it
            # produces (denominator copies) are never read
            v_aug = [
                qkvpool.tile([P, NH, P], at_dt, tag=f"vaug{st}", name=f"vaug{st}")
                for st in range(ST16)
            ]
            zT = [qkvpool.tile([P, S], mm_dt, tag=f"zT{m}", name=f"zT{m}") for m in range(2)]

            # ---- const / weight loads ----
            # memset lacks an f32r encoding; write the same bits as f32
            def _ms(ap, val):
                if ap.dtype == mybir.dt.float32r:
                    ap = ap.bitcast(f32)
                nc.vector.memset(ap, val)

            _ms(ones_sb[:], 1.0)
            for st in range(ST16):
                _ms(v_aug[st][:], 1.0)
            # q weights first so the first projection matmuls start ASAP;
            # k/v weights next; everything else is needed much later
            for kc in range(KC8):
                getattr(nc, ldeng).dma_start(
                    out=wq_sb[:, kc * HD : (kc + 1) * HD],
                    in_=wqT[kc * P : (kc + 1) * P, :],
                )
            for kc in range(KC8):
                getattr(nc, ldeng).dma_start(
                    out=wk_sb[:, kc * HD : (kc + 1) * HD],
                    in_=wkT[kc * P : (kc + 1) * P, :],
                )
                getattr(nc, ldeng).dma_start(
                    out=wv_sb[:, kc * HD : (kc + 1) * HD],
                    in_=wvT[kc * P : (kc + 1) * P, :],
                )
            nc.sync.dma_start(out=bq_sb[:], in_=bq[:])
            nc.sync.dma_start(out=bk_sb[:], in_=bk[:])
            getattr(nc, ldeng).dma_start(out=bv_sb[:], in_=bv[:])
            nc.sync.dma_start(out=mask_sb[:], in_=maskc[:])
            for kc2 in range(2):
                getattr(nc, ldeng).dma_start(
                    out=wo_sb[:, kc2 * D_MODEL : (kc2 + 1) * D_MODEL],
                    in_=woT[kc2 * P : (kc2 + 1) * P, :],
                )

            # ---- phase 1: projections ----
            with (
                nc.named_scope("p1"),
                tc.tile_pool(name="xin", bufs=6) as xin,
                tc.tile_pool(name="ps1", bufs=2, space="PSUM") as ps1,
                tc.tile_pool(name="psv", bufs=4, space="PSUM") as psv,
            ):
                for nh in range(2):
                    nsl = slice(nh * 1024, (nh + 1) * 1024)
                    # q projection (transposed): qT = wq @ x.T
                    pq = [ps1.tile([P, 1024], f32, tag="p1", name="p1") for _ in range(2)]
                    for kc in range(KC8):
                        xc = xin.tile([P, 1024], mm_dt, tag="xc", name="xc")
                        getattr(nc, ldeng).dma_start(
                            out=xc[:], in_=xT[kc * P : (kc + 1) * P, nsl]
                        )
                        for m in range(2):
                            lhs = wq_sb[:, kc * HD + m * P : kc * HD + (m + 1) * P]
                            for n in range(2):
                                nc.tensor.matmul(
                                    pq[m][:, n * 512 : (n + 1) * 512],
                                    r(lhs),
                                    r(xc[:, n * 512 : (n + 1) * 512]),
                                    start=(kc == 0),
                                    stop=(kc == KC8 - 1),
                                )
                    for m in range(2):
                        nc.vector.tensor_scalar_add(
                            qT[m][:, nsl], pq[m][:], bq_sb[:, m : m + 1]
                        )
                    # k (transposed) and v (natural) projections from y
                    pk = [ps1.tile([P, 1024], f32, tag="p1", name="p1") for _ in range(2)]
                    pv = [psv.tile([P, 512], f32, tag="pv", name="pv") for _ in range(4)]
                    for kc in range(KC8):
                        yc = xin.tile([P, 1024], mm_dt, tag="xc", name="xc")
                        getattr(nc, ldeng).dma_start(
                            out=yc[:], in_=yT[kc * P : (kc + 1) * P, nsl]
                        )
                        for m in range(2):
                            lhs = wk_sb[:, kc * HD + m * P : kc * HD + (m + 1) * P]
                            for n in range(2):
                                nc.tensor.matmul(
                                    pk[m][:, n * 512 : (n + 1) * 512],
                                    r(lhs),
                                    r(yc[:, n * 512 : (n + 1) * 512]),
                                    start=(kc == 0),
                                    stop=(kc == KC8 - 1),
                                )
                        for sti in range(8):
                            # two st tiles share one PSUM bank; start=True
                            # clears the WHOLE bank, so only the first tile's
                            # first matmul may carry it (the second tile's
                            # first write lands on cleared has_written bits
                            # and overwrites).
                            nc.tensor.matmul(
                                pv[sti // 2][:, (sti % 2) * 256 : (sti % 2 + 1) * 256],
                                r(yc[:, sti * P : (sti + 1) * P]),
                                r(wv_sb[:, kc * HD : (kc + 1) * HD]),
                                start=(kc == 0 and sti % 2 == 0),
                                stop=False,
                                skip_group_check=True,
                            )
                    for m in range(2):
                        nc.vector.tensor_scalar_add(
                            kT[m][:, nsl], pk[m][:], bk_sb[:, m : m + 1]
                        )
                    for sti in range(8):
                        st = nh * 8 + sti
                        psl = pv[sti // 2][:, (sti % 2) * 256 : (sti % 2 + 1) * 256]
                        # add bias via K=1 matmul (bias varies along free dim)
                        nc.tensor.matmul(
                            psl, ones_sb[:], bv_sb[:], start=False, stop=True
                        )
                        for h in range(NH):
                            nc.vector.tensor_copy(
                                v_aug[st][:, h, 0:D_K],
                                psl[:, h * D_K : (h + 1) * D_K],
                            )

            # ---- phase 2: attention ----
            with (
                nc.named_scope("attn"),
                tc.tile_pool(name="spool", bufs=2, space="PSUM") as spool,
                tc.tile_pool(name="zpool", bufs=2, space="PSUM") as zpool,
                tc.tile_pool(name="apool", bufs=3) as apool,
                tc.tile_pool(name="rows", bufs=2) as rows,
                tc.tile_pool(name="rpool", bufs=2) as rpool,
                tc.tile_pool(name="ztpool", bufs=2) as ztpool,
            ):
                def scores_pair(heads, qh, kc, ps_map):
                    # the two heads' K=64 stationaries sit at partitions 0-63 /
                    # 64-127; explicit tile_position puts them in disjoint PE
                    # row groups so alternating matmuls run concurrently
                    for n in range(2):
                        for h in heads:
                            mb, mo = h // 2, (h % 2) * D_K
                            nc.tensor.matmul(
                                ps_map[h][:, n * 512 : (n + 1) * 512],
                                r(kT[mb][mo : mo + D_K, kc * P : (kc + 1) * P]),
                                r(qT[mb][mo : mo + D_K, qh * 1024 + n * 512 : qh * 1024 + (n + 1) * 512]),
                                start=True,
                                stop=True,
                                tile_position=(mo, 0),
                            )

                # Two heads of a pair are processed interleaved so PE and ACT
                # pipeline; their score matmuls are emitted adjacently and use
                # disjoint PE row groups (K=64 at partitions 0/64), so they
                # run concurrently in the array.
                for qh in range(2):
                    for hp in range(2):
                        heads = (2 * hp, 2 * hp + 1)
                        pz, ps_cur = {}, {}
                        for h in heads:
                            pz[h] = zpool.tile([P, 1024], f32, tag="pz", name="pz")
                        for h in heads:
                            ps_cur[h] = spool.tile([P, 1024], f32, tag="ps", name="ps")
                        scores_pair(heads, qh, 0, ps_cur)

                        def av(h, kc, at):
                            for n in range(2):
                                nc.tensor.matmul(
                                    pz[h][:, n * 512 : (n + 1) * 512],
                                    r(v_aug[kc][:, h, :]),
                                    r(at[:, n * 512 : (n + 1) * 512]),
                                    start=(kc == 0),
                                    stop=(kc == ST16 - 1),
                                )

                        # Round structure: exps first, then the PREVIOUS
                        # round's AVs (they hide under the exps on PE), then
                        # the interleaved scores quad for the next round.
                        prev_attn = None
                        for kc in range(ST16):
                            attn = {}
                            for h in heads:
                                attn[h] = apool.tile(
                                    [P, 1024], at_dt, tag="attn", name="attn"
                                )
                                nc.scalar.activation(
                                    attn[h][:],
                                    ps_cur[h][:],
                                    AF.Exp,
                                    bias=mask_sb[:, kc : kc + 1],
                                    scale=0.125,
                                )
                            if prev_attn is not None:
                                for h in heads:
                                    av(h, kc - 1, prev_attn[h])
                            if kc + 1 < ST16:
                                for h in heads:
                                    ps_cur[h] = spool.tile(
                                        [P, 1024], f32, tag="ps", name="ps"
                                    )
                                scores_pair(heads, qh, kc + 1, ps_cur)
                            prev_attn = attn
                        for h in heads:
                            av(h, ST16 - 1, prev_attn[h])
                        # normalization: z/denominator. First evacuate psum to
                        # SBUF with one copy (frees the z banks fast), then
                        # reciprocal + broadcast + multiply off the hot path.
                        qsl = slice(qh * 1024, (qh + 1) * 1024)
                        for h in heads:
                            mb = h // 2
                            z_sb = ztpool.tile(
                                [D_K + 1, 1024], f32, tag="zsb", name="zsb"
                            )
                            nc.vector.tensor_copy(z_sb[:], pz[h][0 : D_K + 1, :])
                            # the custom DVE op mis-reads inputs at partition
                            # offset 64; re-base the denominator row to
                            # partition 0 first
                            d_row = rows.tile([1, 1024], f32, tag="dr", name="dr")
                            nc.vector.tensor_copy(d_row[:], z_sb[D_K : D_K + 1, :])
                            r_row = rows.tile([1, 1024], f32, tag="rr", name="rr")
                            nc.vector.reciprocal_approx_fast(r_row[:], d_row[:])
                            r_sb = rpool.tile([P, 1024], f32, tag="rb", name="rb")
                            nc.gpsimd.partition_broadcast(
                                r_sb[0:D_K, :], r_row[:], channels=D_K
                            )
                            if h % 2 == 0:
                                nc.vector.tensor_mul(
                                    zT[mb][0:D_K, qsl], z_sb[0:D_K, :], r_sb[0:D_K, :]
                                )
                            else:
                                zt = ztpool.tile(
                                    [D_K, 1024], mm_dt, tag="zt", name="zt"
                                )
                                nc.vector.tensor_mul(
                                    zt[:], z_sb[0:D_K, :], r_sb[0:D_K, :]
                                )
                                nc.sync.dma_start(out=zT[mb][D_K:P, qsl], in_=zt[:])

            # ---- phase 3: output projection (partial; host sums groups) ----
            with (
                nc.named_scope("oproj"),
                tc.tile_pool(name="opool", bufs=4, space="PSUM") as opool,
                tc.tile_pool(name="obuf", bufs=3) as obuf,
            ):
                for st in range(ST16):
                    osb = obuf.tile([P, D_MODEL], f32, tag="osb", name="osb")
                    for n in range(2):
                        po = opool.tile([P, 512], f32, tag="po", name="po")
                        for kc2 in range(2):
                            nc.tensor.matmul(
                                po[:],
                                r(zT[kc2][:, st * P : (st + 1) * P]),
                                r(wo_sb[:, kc2 * D_MODEL + n * 512 : kc2 * D_MODEL + (n + 1) * 512]),
                                start=(kc2 == 0),
                                stop=(kc2 == 1),
                            )
                        # alternate evacuation between DVE and ACT so the tail
                        # isn't single-engine bound
                        osl = osb[:, n * 512 : (n + 1) * 512]
                        if (2 * st + n) % 2 == 0:
                            nc.vector.tensor_copy(osl, po[:])
                        else:
                            nc.scalar.copy(osl, po[:])
                        nc.sync.dma_start(
                            out=out[st * P : (st + 1) * P, n * 512 : (n + 1) * 512],
                            in_=osl,
                        )
            if debug:
                for m in range(2):
                    nc.gpsimd.dma_start(out=dbg["qTo"][m * P : (m + 1) * P, :], in_=qT[m][:])
                    nc.gpsimd.dma_start(out=dbg["kTo"][m * P : (m + 1) * P, :], in_=kT[m][:])
                    nc.gpsimd.dma_start(out=dbg["zTo"][m * P : (m + 1) * P, :], in_=zT[m][:])
                for st in range(ST16):
                    nc.gpsimd.dma_start(
                        out=dbg["vo"][st * P : (st + 1) * P, :],
                        in_=v_aug[st][:].rearrange("p a b -> p (a b)"),
                    )

    nc.compile()
    return nc


def _get_nc():
    if "nc" not in _CACHE:
        _CACHE["nc"] = _build_nc()
    return _CACHE["nc"]


def _ensure_ntff_hook():
    """Provide antenv.axon_hooks (absent in this container) so that
    run_bass_kernel_spmd(trace=True) can NTFF-profile via ctypes."""
    import sys
    import types

    try:
        from antenv.axon_hooks import get_axon_ntff_profile_hook  # noqa: F401

        return
    except ImportError:
        pass
    try:
        from trn_agent_boot.trn_boot import _ntff_profile_via_ctypes

        hook = _ntff_profile_via_ctypes("/opt/axon/libaxon_pjrt.so")
    except Exception:
        hook = None
    mod = types.ModuleType("antenv.axon_hooks")
    state = {"hook": hook}
    mod.get_axon_ntff_profile_hook = lambda: state["hook"]
    mod.set_axon_ntff_profile_hook = lambda h: state.__setitem__("hook", h)
    sys.modules["antenv.axon_hooks"] = mod
    import antenv

    antenv.axon_hooks = mod


def kernel(x, y, mask, wq, bq, wk, bk, wv, bv, wo, bo):
    global last_exec_time_ns, last_results
    from concourse.bass_utils import run_bass_kernel_spmd

    if PROFILE:
        _ensure_ntff_hook()

    x = np.asarray(x, dtype=np.float32)
    y = np.asarray(y, dtype=np.float32)
    mask = np.asarray(mask, dtype=np.float32)
    wq, bq = np.asarray(wq, np.float32), np.asarray(bq, np.float32)
    wk, bk = np.asarray(wk, np.float32), np.asarray(bk, np.float32)
    wv, bv = np.asarray(wv, np.float32), np.asarray(bv, np.float32)
    wo, bo = np.asarray(wo, np.float32), np.asarray(bo, np.float32)

    nc = _get_nc()

    xTs = [np.ascontiguousarray(x[b].T) for b in range(B)]
    yTs = [np.ascontiguousarray(y[b].T) for b in range(B)]
    maskcs = [
        np.ascontiguousarray(((1.0 - mask[b]) * -10000.0).reshape(ST16, P).T)
        for b in range(B)
    ]
    in_maps = []
    for c in range(N_CORES):
        b, g = c // 4, c % 4
        sl = slice(g * HD, (g + 1) * HD)
        in_maps.append(
            {
                "xT": xTs[b],
                "yT": yTs[b],
                "wqT": np.ascontiguousarray(wq[sl, :].T),
                "wkT": np.ascontiguousarray(wk[sl, :].T),
                "wvT": np.ascontiguousarray(wv[sl, :].T),
                "woT": np.ascontiguousarray(wo[:, sl].T),
                "bq": np.ascontiguousarray(bq[sl].reshape(2, P).T),
                "bk": np.ascontiguousarray(bk[sl].reshape(2, P).T),
                "bv": np.ascontiguousarray(bv[sl].reshape(1, HD)),
                "maskc": maskcs[b],
            }
        )

    kwargs = {}
    if PROFILE:
        kwargs["trace"] = True
        if TRACE_DIR:
            os.makedirs(TRACE_DIR, exist_ok=True)
            kwargs["tmpdir"] = TRACE_DIR
    res = run_bass_kernel_spmd(nc, in_maps, list(range(N_CORES)), **kwargs)
    last_results = res
    last_exec_time_ns = res.exec_time_ns

    out = np.empty((B, S, D_MODEL), np.float32)
    for b in range(B):
        acc = res.results[b * 4]["out"].astype(np.float32)
        for g in range(1, 4):
            acc = acc + res.results[b * 4 + g]["out"]
        out[b] = acc + bo[None, :]
    return out
